# revision 52
# baseline (speedup 1.0000x reference)
"""AttentionConv kernel for Trainium2 (8 NeuronCores, SPMD data-parallel over batch).

Problem: per-channel windowed softmax attention.
  q = wq @ x; k = wk @ pad(x, 3); v = wv @ pad(x, 3)       (1x1 convs = GEMMs)
  s_j[c,w] = q[c,w] * k[c,w+j],  j = 0..6
  out[c,w] = sum_j softmax_j(s)[c,w,j] * v[c,w+j]

Sharding: batch B=8 -> one batch element per core; weights replicated.
Since pad commutes with the channel-mixing GEMM, k/v are computed on the
unpadded x and written into SBUF buffers with 3 zero columns on each side.

Per-core engine mapping (v2 — window sums moved to the tensor engine):
  TensorE: 3 GEMMs (256x256 @ 256x4096) into fp32 PSUM, plus the softmax
           numerator/denominator j-sums as 7-deep accumulating matmuls with
           an identity stationary (den = sum_j e_j, num = sum_j e_j*v_j),
           which frees ~45% of the vector engine.
  ScalarE: PSUM->SBUF evacuation casts and exp.
  VectorE: windowed score mult, e*v mult, 1/den via the fast custom-DVE
           reciprocal (fp32), final out = num * rden.
The work is software-pipelined over 16 sub-chunks of 512 columns so DVE
(~5.2us), ACT (~4.8us) and PE (~4.4us) overlap per round.
"""

import sys

sys.path.insert(0, "/opt/trn_rl_repo")

import numpy as np

B, C, W = 8, 256, 4096
K7, PAD = 7, 3
SUB = 512  # attention pipeline sub-chunk (den/num PSUM bank)
GRP = 1024  # gemm group columns (one psum tile, 2 banks)
NSUB = W // SUB  # 8 per co block
NGRP = W // GRP  # 4 per co block
WP = W + 2 * PAD

_STATE = {}

# subs whose rden runs on ACT as exp(-ln(den)) instead of the DVE
# reciprocal — uses ~4us of ACT queue slack to take ~0.7us/sub off the
# DVE stream, which paces the kernel
RDEN_ACT_SUBS = (3, 8, 13)


def _patch_act_tables():
    """Force Exp and Ln to resolve to the one ACT table set containing both,
    so the kernel pays a single ACT_TABLE_LOAD instead of thrashing."""
    import concourse.bacc as bacc_mod
    from concourse import mybir
    from concourse.hw_specs import get_activation_tables as orig

    AF = mybir.ActivationFunctionType

    def patched(arch):
        out = {}
        for name, funcs in orig(arch).items():
            f = set(funcs)
            if name != "natural_log_exp_and_others":
                f.discard(AF.Exp)
                f.discard(AF.Ln)
            out[name] = f
        return out

    bacc_mod.get_activation_tables = patched


def _build_nc():
    import concourse.bass as bass
    import concourse.tile as tile
    from concourse import bacc, mybir

    bf16 = mybir.dt.bfloat16
    f32 = mybir.dt.float32
    AF = mybir.ActivationFunctionType

    _patch_act_tables()

    nc = bacc.Bacc("TRN2", target_bir_lowering=False, debug=False, num_devices=8)

    x_d = nc.declare_dram_parameter("x", [C, W], bf16, isOutput=False)
    w_d = {
        t: nc.declare_dram_parameter(f"wt{t}", [C, C], bf16, isOutput=False)
        for t in "qkv"
    }
    id_d = nc.declare_dram_parameter("ident", [128, 128], bf16, isOutput=False)
    out_d = nc.declare_dram_parameter("out", [C, W], bf16, isOutput=True)

    with tile.TileContext(nc) as tc:
        from contextlib import ExitStack

        with ExitStack() as ctx:
            persist = ctx.enter_context(tc.tile_pool(name="persist", bufs=1))
            gpsum = ctx.enter_context(tc.tile_pool(name="gpsum", bufs=2, space="PSUM"))
            dpsum = ctx.enter_context(tc.tile_pool(name="dpsum", bufs=2, space="PSUM"))
            npsum = ctx.enter_context(tc.tile_pool(name="npsum", bufs=2, space="PSUM"))
            spool = ctx.enter_context(tc.tile_pool(name="spool", bufs=3))
            epool = ctx.enter_context(tc.tile_pool(name="epool", bufs=3))
            rpool = ctx.enter_context(tc.tile_pool(name="rpool", bufs=3))
            opool = ctx.enter_context(tc.tile_pool(name="opool", bufs=3))

            # ---- persistent SBUF tensors ----
            xb = persist.tile([128, 2, W], bf16, tag="xb")
            wsb = {
                t: persist.tile([128, 2, C], bf16, name=f"wsb_{t}", tag=f"wsb_{t}")
                for t in "qkv"
            }  # w.T, ci-major blocks
            qsb = persist.tile([128, 2, W], bf16, tag="qsb")
            ksb = persist.tile([128, 2, WP], bf16, tag="ksb")
            vsb = persist.tile([128, 2, WP], bf16, tag="vsb")
            idb = persist.tile([128, 128], bf16, tag="idb")
            warm = persist.tile([128, 1], bf16, tag="warm")
            # never written: PE warmup matmuls read garbage from here so they
            # have no DMA dependency and start at queue-preamble end (~t6.2)
            junk = persist.tile([128, 128], bf16, tag="junk")

            # ---- loads. Critical path (Sync queue): ident -> wq -> x piece 0,
            # which gates the first gemm. The rest (wk/wv, x pieces 1-3) is
            # triggered from the Scalar queue, idle until the first evac, so
            # the Sync queue reaches the x0 trigger ~4us sooner. ----
            # x piece 0 is the critical load: split it into 512-col halves
            # across BOTH HWDGE queue groups (sync + scalar). The gemm's
            # h0 matmuls only depend on the first half, so they start ~3us
            # before the full piece lands.
            nc.sync.dma_start(out=xb[:, 0, 0:512], in_=x_d[0:128, 0:512])
            nc.sync.dma_start(out=xb[:, 1, 0:512], in_=x_d[128:256, 0:512])
            nc.sync.dma_start(out=xb[:, 0, 512:GRP], in_=x_d[0:128, 512:GRP])
            nc.sync.dma_start(out=xb[:, 1, 512:GRP], in_=x_d[128:256, 512:GRP])
            for cb in range(2):
                nc.sync.dma_start(
                    out=wsb["q"][:, cb, :], in_=w_d["q"][cb * 128 : (cb + 1) * 128, :]
                )
            nc.sync.dma_start(out=idb[:, :], in_=id_d[:, :])
            # x cols 1024:2048 gate group 1 (needed by scores_1 at ~t17):
            # own early pair, split across both queue groups.
            nc.sync.dma_start(out=xb[:, 0, GRP : 2 * GRP], in_=x_d[0:128, GRP : 2 * GRP])
            nc.scalar.dma_start(
                out=xb[:, 1, GRP : 2 * GRP], in_=x_d[128:256, GRP : 2 * GRP]
            )
            # x cols 2048:4096 are 4KB-contiguous per row (fast packets) and
            # not needed until ~t33.
            for cb in range(2):
                nc.sync.dma_start(
                    out=xb[:, cb, 2 * GRP : 4 * GRP],
                    in_=x_d[cb * 128 : (cb + 1) * 128, 2 * GRP : 4 * GRP],
                )

            # zero the ACT warmup tile before the exp below reads it, and
            # the PE warmup tile (cheap gpsimd write, no DMA dependency)
            nc.gpsimd.memset(warm[:, :], 0.0)
            nc.gpsimd.memset(junk[:, :], 0.0)

            # First ACT instruction is a tiny exp so the one table set
            # (exp + copy) loads at t~7us, off the critical path.
            nc.scalar.activation(warm[:, :], warm[:, :], AF.Exp)

            for t in "kv":
                for cb in range(2):
                    nc.scalar.dma_start(
                        out=wsb[t][:, cb, :], in_=w_d[t][cb * 128 : (cb + 1) * 128, :]
                    )


            # zero the pad columns of k/v (gpsimd keeps these off the busy
            # engines)
            for buf in (ksb, vsb):
                for cb in range(2):
                    nc.gpsimd.memset(buf[:, cb, 0:PAD], 0.0)
                    nc.gpsimd.memset(buf[:, cb, W + PAD : WP], 0.0)

            # PE warmup on the identity tiles: the HAM clock state follows
            # sustained matmul activity, and the steady-state matmul rate for
            # the whole kernel depends on it (measured 216 vs 259 ns/matmul).
            # Split the burst so the first gemm group is not delayed: 8
            # matmuls start the ramp while the x DMA lands; 16 more are
            # emitted right after the first group (see below) to finish it.
            wps = gpsum.tile([128, 128], f32, name="wps", tag="g")

            def pe_warmup(n):
                for i in range(n):
                    nc.tensor.matmul(
                        wps[:, :],
                        junk[:, :],
                        junk[:, :],
                        start=True,
                        stop=True,
                        skip_group_check=True,
                    )

            pe_warmup(8)

            emitted = [0, 0]  # gemm groups emitted per co block
            v_queue = []  # (co, g) groups whose v gemm is deferred one round

            def gemm_tensor(co, g, t):
                """One tensor's GEMM for cols [g*GRP, (g+1)*GRP) of co."""
                co_sl = slice(co * 128, (co + 1) * 128)
                c0 = g * GRP
                ps = gpsum.tile([128, GRP], f32, name=f"g{co}{g}{t}", tag="g")
                for h in range(GRP // 512):
                    for ci in range(2):
                        nc.tensor.matmul(
                            ps[:, h * 512 : (h + 1) * 512],
                            wsb[t][:, ci, co_sl],
                            xb[:, ci, c0 + h * 512 : c0 + (h + 1) * 512],
                            start=(ci == 0),
                            stop=(ci == 1),
                        )
                if t == "q":
                    dst = qsb[:, co, c0 : c0 + GRP]
                else:
                    buf = ksb if t == "k" else vsb
                    dst = buf[:, co, PAD + c0 : PAD + c0 + GRP]
                nc.scalar.copy(out=dst, in_=ps[:, :])

            def need_gemms(co, upto):
                """Emit q and k gemms up to group `upto`; v is queued and
                emitted one round later — it is only needed by ev, ~5us
                after scores, and its 4 matmuls otherwise sit on the
                critical PE path to the next scores' k evacuation."""
                for g in range(emitted[co], min(upto, NGRP - 1) + 1):
                    gemm_tensor(co, g, "q")
                    gemm_tensor(co, g, "k")
                    v_queue.append((co, g))
                emitted[co] = max(emitted[co], min(upto, NGRP - 1) + 1)

            def flush_one_v():
                if v_queue:
                    co, g = v_queue.pop(0)
                    gemm_tensor(co, g, "v")

            def sub_scores(k):
                """scores + exp for flat sub-chunk k; returns the e tile.
                Edge subs (first/last) split in j-halves so exp starts ~1.1us
                into scores and the den matmuls start per-row — shortens the
                pipeline fill and drain without steady-state overhead."""
                co, ch = k // NSUB, k % NSUB
                w0 = ch * SUB
                s = spool.tile([128, K7, SUB], bf16, name=f"s{k}", tag="s")
                qsl = qsb[:, co, w0 : w0 + SUB]
                ksl = ksb[:, co, w0 : w0 + SUB]

                def q_bc(j0, n):
                    return bass.AP(
                        tensor=qsl.tensor,
                        offset=qsl.offset,
                        ap=[qsl.ap[0], [0, n], [1, SUB]],
                    )

                def k_wn(j0, n):
                    return bass.AP(
                        tensor=ksl.tensor,
                        offset=ksl.offset + j0,
                        ap=[ksl.ap[0], [1, n], [1, SUB]],
                    )

                if k == 0 or k == NTOT - 1:
                    nc.vector.tensor_mul(s[:, 0:4, :], q_bc(0, 4), k_wn(0, 4))
                    nc.scalar.activation(s[:, 0:4, :], s[:, 0:4, :], AF.Exp)
                    nc.vector.tensor_mul(s[:, 4:K7, :], q_bc(4, 3), k_wn(4, 3))
                    nc.scalar.activation(s[:, 4:K7, :], s[:, 4:K7, :], AF.Exp)
                else:
                    nc.vector.tensor_mul(s[:, :, :], q_bc(0, K7), k_wn(0, K7))
                    nc.scalar.activation(s[:, :, :], s[:, :, :], AF.Exp)
                return s

            def sub_den(k, s):
                """den = sum_j e_j : 7 accumulating identity matmuls (PE)."""
                den = dpsum.tile([128, SUB], f32, name=f"d{k}", tag="d")
                for j in range(K7):
                    nc.tensor.matmul(
                        den[:, :],
                        idb[:, :],
                        s[:, j, :],
                        start=(j == 0),
                        stop=(j == K7 - 1),
                    )
                return den

            def sub_mid(k, s, den):
                """ev (DVE), rden (DVE recip), num (PE) for sub k."""
                co, ch = k // NSUB, k % NSUB
                w0 = ch * SUB
                # ev = e * v_window (separate tile: PE reads s concurrently)
                ev = epool.tile([128, K7, SUB], bf16, name=f"e{k}", tag="e")
                vsl = vsb[:, co, w0 : w0 + SUB]
                v_w = bass.AP(
                    tensor=vsl.tensor,
                    offset=vsl.offset,
                    ap=[vsl.ap[0], [1, K7], [1, SUB]],
                )
                if k == 0 or k == NTOT - 1:
                    v_wa = bass.AP(
                        tensor=vsl.tensor,
                        offset=vsl.offset,
                        ap=[vsl.ap[0], [1, 4], [1, SUB]],
                    )
                    v_wb = bass.AP(
                        tensor=vsl.tensor,
                        offset=vsl.offset + 4,
                        ap=[vsl.ap[0], [1, 3], [1, SUB]],
                    )
                    nc.vector.tensor_mul(ev[:, 0:4, :], v_wa, s[:, 0:4, :])
                    nc.vector.tensor_mul(ev[:, 4:K7, :], v_wb, s[:, 4:K7, :])
                else:
                    nc.vector.tensor_mul(ev[:, :, :], v_w, s[:, :, :])
                # rden = 1/den: DVE fast reciprocal normally; for a few subs
                # use ACT exp(-ln(den)) to shift work into ACT queue slack
                rden = rpool.tile([128, SUB], f32, name=f"r{k}", tag="r")
                if k in RDEN_ACT_SUBS:
                    nc.scalar.activation(rden[:, :], den[:, :], AF.Ln)
                    nc.scalar.activation(rden[:, :], rden[:, :], AF.Exp, scale=-1.0)
                else:
                    nc.vector.reciprocal_approx_fast(out=rden[:, :], in_=den[:, :])
                # num = sum_j ev_j
                num = npsum.tile([128, SUB], f32, name=f"n{k}", tag="n")
                for j in range(K7):
                    nc.tensor.matmul(
                        num[:, :],
                        idb[:, :],
                        ev[:, j, :],
                        start=(j == 0),
                        stop=(j == K7 - 1),
                    )
                return rden, num

            def sub_final(k, rden, num):
                co, ch = k // NSUB, k % NSUB
                w0 = ch * SUB
                co_sl = slice(co * 128, (co + 1) * 128)
                oc = opool.tile([128, SUB], bf16, name=f"o{k}", tag="o")
                nc.vector.tensor_mul(oc[:, :], num[:, :], rden[:, :])
                nc.sync.dma_start(out=out_d[co_sl, w0 : w0 + SUB], in_=oc[:, :])

            NTOT = 2 * NSUB  # 16 flat sub-chunks, co-major
            live = {}  # k -> (s tile, den) then (rden, num)
            need_gemms(0, 0)

            # Finish the HAM ramp: 16 more warmup matmuls right after the
            # first gemm group, into a den-pool tile (recycled long before
            # den_1 needs the slot).
            wps2 = dpsum.tile([128, SUB], f32, name="wps2", tag="d")
            for i in range(16):
                nc.tensor.matmul(
                    wps2[:, 0:128],
                    junk[:, :],
                    junk[:, :],
                    start=True,
                    stop=True,
                    skip_group_check=True,
                )
            # Round r: gemm prefetch for sub r+1 first (evacs land on ACT
            # ahead of exp_r), then scores_r + exp_r feed the pipe, then the
            # mid stages of sub r-1 and the final of sub r-2 fill the round.
            for r in range(NTOT + 2):
                # gemm prefetch first: those matmuls are ready to run, while
                # den of sub r-1 waits on exp_{r-1} — den at the PE queue
                # head would block them (measured ~2us worse).
                if r + 1 < NTOT:
                    co2, ch2 = (r + 1) // NSUB, (r + 1) % NSUB
                    need_gemms(co2, (ch2 + 1) // 2)
                flush_one_v()
                if r < NTOT:
                    live[r] = [sub_scores(r), None]
                if 1 <= r <= NTOT:
                    k = r - 1
                    den = sub_den(k, live[k][0])
                    live[k] = sub_mid(k, live[k][0], den)
                if 2 <= r:
                    k = r - 2
                    rden, num = live.pop(k)
                    sub_final(k, rden, num)

    nc.finalize()
    return nc


def _get_nc():
    if "nc" not in _STATE:
        _STATE["nc"] = _build_nc()
    return _STATE["nc"]


def make_in_maps(x, wq, wk, wv):
    import ml_dtypes

    bf = ml_dtypes.bfloat16
    x = np.asarray(x, dtype=np.float32)
    wqT = np.ascontiguousarray(np.asarray(wq, dtype=np.float32).T).astype(bf)
    wkT = np.ascontiguousarray(np.asarray(wk, dtype=np.float32).T).astype(bf)
    wvT = np.ascontiguousarray(np.asarray(wv, dtype=np.float32).T).astype(bf)
    xb = x.astype(bf)
    ident = np.eye(128, dtype=np.float32).astype(bf)
    return [
        {
            "x": np.ascontiguousarray(xb[b]),
            "wtq": wqT,
            "wtk": wkT,
            "wtv": wvT,
            "ident": ident,
        }
        for b in range(B)
    ]


def kernel(x, wq, wk, wv):
    nc = _get_nc()
    in_maps = make_in_maps(x, wq, wk, wv)

    from concourse.bass_utils import run_bass_kernel_spmd

    res = run_bass_kernel_spmd(nc, in_maps, core_ids=list(range(B)))
    outs = [np.asarray(res.results[i]["out"], dtype=np.float32) for i in range(B)]
    return np.stack(outs)


# revision 55
# speedup vs baseline: 1.0003x; 1.0003x over previous
"""AttentionConv kernel for Trainium2 (8 NeuronCores, SPMD data-parallel over batch).

Problem: per-channel windowed softmax attention.
  q = wq @ x; k = wk @ pad(x, 3); v = wv @ pad(x, 3)       (1x1 convs = GEMMs)
  s_j[c,w] = q[c,w] * k[c,w+j],  j = 0..6
  out[c,w] = sum_j softmax_j(s)[c,w,j] * v[c,w+j]

Sharding: batch B=8 -> one batch element per core; weights replicated.
Since pad commutes with the channel-mixing GEMM, k/v are computed on the
unpadded x and written into SBUF buffers with 3 zero columns on each side.

Per-core engine mapping (v2 — window sums moved to the tensor engine):
  TensorE: 3 GEMMs (256x256 @ 256x4096) into fp32 PSUM, plus the softmax
           numerator/denominator j-sums as 7-deep accumulating matmuls with
           an identity stationary (den = sum_j e_j, num = sum_j e_j*v_j),
           which frees ~45% of the vector engine.
  ScalarE: PSUM->SBUF evacuation casts and exp.
  VectorE: windowed score mult, e*v mult, 1/den via the fast custom-DVE
           reciprocal (fp32), final out = num * rden.
The work is software-pipelined over 16 sub-chunks of 512 columns so DVE
(~5.2us), ACT (~4.8us) and PE (~4.4us) overlap per round.
"""

import sys

sys.path.insert(0, "/opt/trn_rl_repo")

import numpy as np

B, C, W = 8, 256, 4096
K7, PAD = 7, 3
SUB = 512  # attention pipeline sub-chunk (den/num PSUM bank)
GRP = 1024  # gemm group columns (one psum tile, 2 banks)
NSUB = W // SUB  # 8 per co block
NGRP = W // GRP  # 4 per co block
WP = W + 2 * PAD

_STATE = {}


def _build_nc():
    import concourse.bass as bass
    import concourse.tile as tile
    from concourse import bacc, mybir

    bf16 = mybir.dt.bfloat16
    f32 = mybir.dt.float32
    AF = mybir.ActivationFunctionType

    nc = bacc.Bacc("TRN2", target_bir_lowering=False, debug=False, num_devices=8)

    x_d = nc.declare_dram_parameter("x", [C, W], bf16, isOutput=False)
    w_d = {
        t: nc.declare_dram_parameter(f"wt{t}", [C, C], bf16, isOutput=False)
        for t in "qkv"
    }
    id_d = nc.declare_dram_parameter("ident", [128, 128], bf16, isOutput=False)
    out_d = nc.declare_dram_parameter("out", [C, W], bf16, isOutput=True)

    with tile.TileContext(nc) as tc:
        from contextlib import ExitStack

        with ExitStack() as ctx:
            persist = ctx.enter_context(tc.tile_pool(name="persist", bufs=1))
            gpsum = ctx.enter_context(tc.tile_pool(name="gpsum", bufs=2, space="PSUM"))
            dpsum = ctx.enter_context(tc.tile_pool(name="dpsum", bufs=2, space="PSUM"))
            npsum = ctx.enter_context(tc.tile_pool(name="npsum", bufs=2, space="PSUM"))
            spool = ctx.enter_context(tc.tile_pool(name="spool", bufs=3))
            epool = ctx.enter_context(tc.tile_pool(name="epool", bufs=3))
            rpool = ctx.enter_context(tc.tile_pool(name="rpool", bufs=3))
            opool = ctx.enter_context(tc.tile_pool(name="opool", bufs=3))

            # ---- persistent SBUF tensors ----
            xb = persist.tile([128, 2, W], bf16, tag="xb")
            wsb = {
                t: persist.tile([128, 2, C], bf16, name=f"wsb_{t}", tag=f"wsb_{t}")
                for t in "qkv"
            }  # w.T, ci-major blocks
            qsb = persist.tile([128, 2, W], bf16, tag="qsb")
            ksb = persist.tile([128, 2, WP], bf16, tag="ksb")
            vsb = persist.tile([128, 2, WP], bf16, tag="vsb")
            idb = persist.tile([128, 128], bf16, tag="idb")
            warm = persist.tile([128, 1], bf16, tag="warm")
            # never written: PE warmup matmuls read garbage from here so they
            # have no DMA dependency and start at queue-preamble end (~t6.2)
            junk = persist.tile([128, 128], bf16, tag="junk")
            # DVE power-warmup scratch: busy-work during the t6-11 DMA wait
            # raises chip activity so the DVFS clock ramps before the first
            # real gemm matmuls
            dummy = persist.tile([128, 4096], bf16, tag="dummy")

            # ---- loads. Critical path (Sync queue): ident -> wq -> x piece 0,
            # which gates the first gemm. The rest (wk/wv, x pieces 1-3) is
            # triggered from the Scalar queue, idle until the first evac, so
            # the Sync queue reaches the x0 trigger ~4us sooner. ----
            # x piece 0 is the critical load: split it into 512-col halves
            # across BOTH HWDGE queue groups (sync + scalar). The gemm's
            # h0 matmuls only depend on the first half, so they start ~3us
            # before the full piece lands.
            nc.sync.dma_start(out=xb[:, 0, 0:512], in_=x_d[0:128, 0:512])
            nc.sync.dma_start(out=xb[:, 1, 0:512], in_=x_d[128:256, 0:512])
            nc.sync.dma_start(out=xb[:, 0, 512:GRP], in_=x_d[0:128, 512:GRP])
            nc.sync.dma_start(out=xb[:, 1, 512:GRP], in_=x_d[128:256, 512:GRP])
            for cb in range(2):
                nc.sync.dma_start(
                    out=wsb["q"][:, cb, :], in_=w_d["q"][cb * 128 : (cb + 1) * 128, :]
                )
            nc.sync.dma_start(out=idb[:, :], in_=id_d[:, :])
            # x cols 1024:2048 gate group 1 (needed by scores_1 at ~t17):
            # own early pair, split across both queue groups.
            nc.sync.dma_start(out=xb[:, 0, GRP : 2 * GRP], in_=x_d[0:128, GRP : 2 * GRP])
            nc.scalar.dma_start(
                out=xb[:, 1, GRP : 2 * GRP], in_=x_d[128:256, GRP : 2 * GRP]
            )
            # x cols 2048:4096 are 4KB-contiguous per row (fast packets) and
            # not needed until ~t33.
            for cb in range(2):
                nc.sync.dma_start(
                    out=xb[:, cb, 2 * GRP : 4 * GRP],
                    in_=x_d[cb * 128 : (cb + 1) * 128, 2 * GRP : 4 * GRP],
                )

            # zero the ACT warmup tile before the exp below reads it, and
            # the PE warmup tile (cheap gpsimd write, no DMA dependency)
            nc.gpsimd.memset(warm[:, :], 0.0)
            nc.gpsimd.memset(junk[:, :], 0.0)

            # DVE power-warmup: ~5.5us of vector activity while the x DMA
            # lands (DVE is otherwise idle until t~19)
            nc.vector.memset(dummy[:, :], 0.0)
            nc.vector.tensor_add(dummy[:, :], dummy[:, :], dummy[:, :])
            nc.vector.tensor_add(dummy[:, :], dummy[:, :], dummy[:, :])

            # First ACT instruction is a tiny exp so the one table set
            # (exp + copy) loads at t~7us, off the critical path.
            nc.scalar.activation(warm[:, :], warm[:, :], AF.Exp)

            for t in "kv":
                for cb in range(2):
                    nc.scalar.dma_start(
                        out=wsb[t][:, cb, :], in_=w_d[t][cb * 128 : (cb + 1) * 128, :]
                    )


            # zero the pad columns of k/v (gpsimd keeps these off the busy
            # engines)
            for buf in (ksb, vsb):
                for cb in range(2):
                    nc.gpsimd.memset(buf[:, cb, 0:PAD], 0.0)
                    nc.gpsimd.memset(buf[:, cb, W + PAD : WP], 0.0)

            # PE warmup on the identity tiles: the HAM clock state follows
            # sustained matmul activity, and the steady-state matmul rate for
            # the whole kernel depends on it (measured 216 vs 259 ns/matmul).
            # Split the burst so the first gemm group is not delayed: 8
            # matmuls start the ramp while the x DMA lands; 16 more are
            # emitted right after the first group (see below) to finish it.
            wps = gpsum.tile([128, 128], f32, name="wps", tag="g")

            def pe_warmup(n):
                for i in range(n):
                    nc.tensor.matmul(
                        wps[:, :],
                        junk[:, :],
                        junk[:, :],
                        start=True,
                        stop=True,
                        skip_group_check=True,
                    )

            pe_warmup(8)

            emitted = [0, 0]  # gemm groups emitted per co block
            v_queue = []  # (co, g) groups whose v gemm is deferred one round

            def gemm_tensor(co, g, t):
                """One tensor's GEMM for cols [g*GRP, (g+1)*GRP) of co."""
                co_sl = slice(co * 128, (co + 1) * 128)
                c0 = g * GRP
                ps = gpsum.tile([128, GRP], f32, name=f"g{co}{g}{t}", tag="g")
                for h in range(GRP // 512):
                    for ci in range(2):
                        nc.tensor.matmul(
                            ps[:, h * 512 : (h + 1) * 512],
                            wsb[t][:, ci, co_sl],
                            xb[:, ci, c0 + h * 512 : c0 + (h + 1) * 512],
                            start=(ci == 0),
                            stop=(ci == 1),
                        )
                if t == "q":
                    dst = qsb[:, co, c0 : c0 + GRP]
                else:
                    buf = ksb if t == "k" else vsb
                    dst = buf[:, co, PAD + c0 : PAD + c0 + GRP]
                nc.scalar.copy(out=dst, in_=ps[:, :])

            def need_gemms(co, upto):
                """Emit q and k gemms up to group `upto`; v is queued and
                emitted one round later — it is only needed by ev, ~5us
                after scores, and its 4 matmuls otherwise sit on the
                critical PE path to the next scores' k evacuation."""
                for g in range(emitted[co], min(upto, NGRP - 1) + 1):
                    gemm_tensor(co, g, "q")
                    gemm_tensor(co, g, "k")
                    v_queue.append((co, g))
                emitted[co] = max(emitted[co], min(upto, NGRP - 1) + 1)

            def flush_one_v():
                if v_queue:
                    co, g = v_queue.pop(0)
                    gemm_tensor(co, g, "v")

            def sub_scores(k):
                """scores + exp for flat sub-chunk k; returns the e tile.
                Edge subs (first/last) split in j-halves so exp starts ~1.1us
                into scores and the den matmuls start per-row — shortens the
                pipeline fill and drain without steady-state overhead."""
                co, ch = k // NSUB, k % NSUB
                w0 = ch * SUB
                s = spool.tile([128, K7, SUB], bf16, name=f"s{k}", tag="s")
                qsl = qsb[:, co, w0 : w0 + SUB]
                ksl = ksb[:, co, w0 : w0 + SUB]

                def q_bc(j0, n):
                    return bass.AP(
                        tensor=qsl.tensor,
                        offset=qsl.offset,
                        ap=[qsl.ap[0], [0, n], [1, SUB]],
                    )

                def k_wn(j0, n):
                    return bass.AP(
                        tensor=ksl.tensor,
                        offset=ksl.offset + j0,
                        ap=[ksl.ap[0], [1, n], [1, SUB]],
                    )

                if k == 0 or k == NTOT - 1:
                    nc.vector.tensor_mul(s[:, 0:4, :], q_bc(0, 4), k_wn(0, 4))
                    nc.scalar.activation(s[:, 0:4, :], s[:, 0:4, :], AF.Exp)
                    nc.vector.tensor_mul(s[:, 4:K7, :], q_bc(4, 3), k_wn(4, 3))
                    nc.scalar.activation(s[:, 4:K7, :], s[:, 4:K7, :], AF.Exp)
                else:
                    nc.vector.tensor_mul(s[:, :, :], q_bc(0, K7), k_wn(0, K7))
                    nc.scalar.activation(s[:, :, :], s[:, :, :], AF.Exp)
                return s

            def sub_den(k, s):
                """den = sum_j e_j : 7 accumulating identity matmuls (PE)."""
                den = dpsum.tile([128, SUB], f32, name=f"d{k}", tag="d")
                for j in range(K7):
                    nc.tensor.matmul(
                        den[:, :],
                        idb[:, :],
                        s[:, j, :],
                        start=(j == 0),
                        stop=(j == K7 - 1),
                    )
                return den

            def sub_mid(k, s, den):
                """ev (DVE), rden (DVE recip), num (PE) for sub k."""
                co, ch = k // NSUB, k % NSUB
                w0 = ch * SUB
                # ev = e * v_window (separate tile: PE reads s concurrently)
                ev = epool.tile([128, K7, SUB], bf16, name=f"e{k}", tag="e")
                vsl = vsb[:, co, w0 : w0 + SUB]
                v_w = bass.AP(
                    tensor=vsl.tensor,
                    offset=vsl.offset,
                    ap=[vsl.ap[0], [1, K7], [1, SUB]],
                )
                if k == 0 or k == NTOT - 1:
                    v_wa = bass.AP(
                        tensor=vsl.tensor,
                        offset=vsl.offset,
                        ap=[vsl.ap[0], [1, 4], [1, SUB]],
                    )
                    v_wb = bass.AP(
                        tensor=vsl.tensor,
                        offset=vsl.offset + 4,
                        ap=[vsl.ap[0], [1, 3], [1, SUB]],
                    )
                    nc.vector.tensor_mul(ev[:, 0:4, :], v_wa, s[:, 0:4, :])
                    nc.vector.tensor_mul(ev[:, 4:K7, :], v_wb, s[:, 4:K7, :])
                else:
                    nc.vector.tensor_mul(ev[:, :, :], v_w, s[:, :, :])
                # rden = 1/den (fast custom-DVE reciprocal, fp32)
                rden = rpool.tile([128, SUB], f32, name=f"r{k}", tag="r")
                nc.vector.reciprocal_approx_fast(out=rden[:, :], in_=den[:, :])
                # num = sum_j ev_j
                num = npsum.tile([128, SUB], f32, name=f"n{k}", tag="n")
                for j in range(K7):
                    nc.tensor.matmul(
                        num[:, :],
                        idb[:, :],
                        ev[:, j, :],
                        start=(j == 0),
                        stop=(j == K7 - 1),
                    )
                return rden, num

            def sub_final(k, rden, num):
                co, ch = k // NSUB, k % NSUB
                w0 = ch * SUB
                co_sl = slice(co * 128, (co + 1) * 128)
                oc = opool.tile([128, SUB], bf16, name=f"o{k}", tag="o")
                nc.vector.tensor_mul(oc[:, :], num[:, :], rden[:, :])
                nc.sync.dma_start(out=out_d[co_sl, w0 : w0 + SUB], in_=oc[:, :])

            NTOT = 2 * NSUB  # 16 flat sub-chunks, co-major
            live = {}  # k -> (s tile, den) then (rden, num)
            need_gemms(0, 0)

            # Finish the HAM ramp: 16 more warmup matmuls right after the
            # first gemm group, into a den-pool tile (recycled long before
            # den_1 needs the slot).
            wps2 = dpsum.tile([128, SUB], f32, name="wps2", tag="d")
            for i in range(16):
                nc.tensor.matmul(
                    wps2[:, 0:128],
                    junk[:, :],
                    junk[:, :],
                    start=True,
                    stop=True,
                    skip_group_check=True,
                )
            # Round r: gemm prefetch for sub r+1 first (evacs land on ACT
            # ahead of exp_r), then scores_r + exp_r feed the pipe, then the
            # mid stages of sub r-1 and the final of sub r-2 fill the round.
            for r in range(NTOT + 2):
                # gemm prefetch first: those matmuls are ready to run, while
                # den of sub r-1 waits on exp_{r-1} — den at the PE queue
                # head would block them (measured ~2us worse).
                if r + 1 < NTOT:
                    co2, ch2 = (r + 1) // NSUB, (r + 1) % NSUB
                    need_gemms(co2, (ch2 + 1) // 2)
                flush_one_v()
                if r < NTOT:
                    live[r] = [sub_scores(r), None]
                if 1 <= r <= NTOT:
                    k = r - 1
                    den = sub_den(k, live[k][0])
                    live[k] = sub_mid(k, live[k][0], den)
                if 2 <= r:
                    k = r - 2
                    rden, num = live.pop(k)
                    sub_final(k, rden, num)

    nc.finalize()
    return nc


def _get_nc():
    if "nc" not in _STATE:
        _STATE["nc"] = _build_nc()
    return _STATE["nc"]


def make_in_maps(x, wq, wk, wv):
    import ml_dtypes

    bf = ml_dtypes.bfloat16
    x = np.asarray(x, dtype=np.float32)
    wqT = np.ascontiguousarray(np.asarray(wq, dtype=np.float32).T).astype(bf)
    wkT = np.ascontiguousarray(np.asarray(wk, dtype=np.float32).T).astype(bf)
    wvT = np.ascontiguousarray(np.asarray(wv, dtype=np.float32).T).astype(bf)
    xb = x.astype(bf)
    ident = np.eye(128, dtype=np.float32).astype(bf)
    return [
        {
            "x": np.ascontiguousarray(xb[b]),
            "wtq": wqT,
            "wtk": wkT,
            "wtv": wvT,
            "ident": ident,
        }
        for b in range(B)
    ]


def kernel(x, wq, wk, wv):
    nc = _get_nc()
    in_maps = make_in_maps(x, wq, wk, wv)

    from concourse.bass_utils import run_bass_kernel_spmd

    res = run_bass_kernel_spmd(nc, in_maps, core_ids=list(range(B)))
    outs = [np.asarray(res.results[i]["out"], dtype=np.float32) for i in range(B)]
    return np.stack(outs)
